# revision 2
# baseline (speedup 1.0000x reference)
"""Trainium2 Bass kernel for nn_ExtendedSympNet (Suzuki-4 composition of
extended symplectic verlet steps driven by a 6-layer MLP Hamiltonian).

Strategy: pure data parallel over 8 NeuronCores (4096 samples each).
Activations are kept feature-major [512 feat (partitions), 512 batch (free)];
each of the 10 gradient evaluations is a fused forward+backward pass of the
MLP done fully on-chip. Matmuls run in float32r (full PE rate at N=512),
the integrator state stays in float32.

Self-contained: hardcodes all shapes from the problem spec.
"""
import numpy as np
from contextlib import ExitStack

import concourse.bacc as bacc
import concourse.bass as bass
import concourse.mybir as mybir
import concourse.tile as tile
from concourse.bass_utils import run_bass_kernel_spmd

F32 = mybir.dt.float32
F32R = mybir.dt.float32r
AF = mybir.ActivationFunctionType
ALU = mybir.AluOpType

B, LAT, HID = 32768, 64, 512
N_CORES = 8
BC = B // N_CORES          # samples per core = 4096
BT = 512                   # batch tile (matmul moving dim / PSUM bank)
NBT = BC // BT             # 8 batch tiles per core
DT = 0.1
NSTEP = 5                  # Suzuki composition sub-steps
NL = 5                     # tanh layers in H-net


def _pack_k(w: np.ndarray) -> np.ndarray:
    """[512, C] -> [128, 4*C]: 128-row k-tile blocks side by side."""
    assert w.shape[0] == 4 * 128
    return np.concatenate([w[k * 128:(k + 1) * 128, :] for k in range(4)], axis=1)


def build_program(n_bt: int = NBT, n_step: int = NSTEP):
    nc = bacc.Bacc("TRN2", target_bir_lowering=False, debug=False)

    # ---- DRAM io ----
    d = {}
    d["zf"] = nc.dram_tensor("zf", [LAT, BC], F32, kind="ExternalInput").ap()
    d["zr"] = nc.dram_tensor("zr", [LAT, BC], F32R, kind="ExternalInput").ap()
    d["w1"] = nc.dram_tensor("w1", [LAT, HID], F32R, kind="ExternalInput").ap()
    # forward weights W2..W5 packed: [128, 4 layers * 2048]
    d["wf"] = nc.dram_tensor("wf", [128, 4 * 4 * HID], F32R, kind="ExternalInput").ap()
    # backward weights W2^T..W5^T packed the same way
    d["wb"] = nc.dram_tensor("wb", [128, 4 * 4 * HID], F32R, kind="ExternalInput").ap()
    # W1[:4,:].T with columns permuted [2,3,0,1], packed: [128, 16]
    d["wga"] = nc.dram_tensor("wga", [128, 16], F32R, kind="ExternalInput").ap()
    d["w6"] = nc.dram_tensor("w6", [128, 4], F32, kind="ExternalInput").ap()
    d["bia"] = nc.dram_tensor("bia", [128, 4 * NL], F32, kind="ExternalInput").ap()
    # per half-step active-update matrices A_hs [4,4] packed: [4, 4*2*NSTEP]
    d["smp"] = nc.dram_tensor("smp", [4, 4 * 2 * NSTEP], F32R, kind="ExternalInput").ap()
    # per half-step gradient coefficient vectors: [4, 2*NSTEP]
    d["cvp"] = nc.dram_tensor("cvp", [4, 2 * NSTEP], F32, kind="ExternalInput").ap()
    zo = nc.dram_tensor("zo", [LAT, BC], F32, kind="ExternalOutput").ap()

    with tile.TileContext(nc) as tc, ExitStack() as ctx:
        wpool = ctx.enter_context(tc.tile_pool(name="wpool", bufs=1))
        hpool = ctx.enter_context(tc.tile_pool(name="hpool", bufs=10))
        tpool = ctx.enter_context(tc.tile_pool(name="tpool", bufs=21))
        dpool = ctx.enter_context(tc.tile_pool(name="dpool", bufs=10))
        gpool = ctx.enter_context(tc.tile_pool(name="gpool", bufs=3))
        ppool = ctx.enter_context(tc.tile_pool(name="ppool", bufs=6, space="PSUM"))
        spool = ctx.enter_context(tc.tile_pool(name="spool", bufs=1, space="PSUM"))

        # ---- persistent SBUF ----
        zf_sb = wpool.tile([LAT, BC], F32)      # integrator state (output)
        zr_sb = wpool.tile([LAT, BC], F32R)     # matmul-operand mirror
        w1_sb = wpool.tile([LAT, HID], F32R)
        wf_sb = wpool.tile([128, 4 * 4 * HID], F32R)
        wb_sb = wpool.tile([128, 4 * 4 * HID], F32R)
        wga_sb = wpool.tile([128, 16], F32R)
        w6_sb = wpool.tile([128, 4], F32)
        bia_sb = wpool.tile([128, 4 * NL], F32)
        smp_sb = wpool.tile([4, 4 * 2 * NSTEP], F32R)
        cvp_sb = wpool.tile([4, 2 * NSTEP], F32)
        for name, t in (("zf", zf_sb), ("zr", zr_sb), ("w1", w1_sb), ("wf", wf_sb),
                        ("wb", wb_sb), ("wga", wga_sb), ("w6", w6_sb), ("bia", bia_sb),
                        ("smp", smp_sb), ("cvp", cvp_sb)):
            nc.sync.dma_start(t[:], d[name][:])

        def grad_active(btsl):
            """Forward+backward through the H-net for one batch tile.
            Returns the PSUM tile [4, BT] holding the SWAPPED active gradient:
            rows 0:2 = dH/dz2, rows 2:4 = dH/dz1 (at zr[:, btsl])."""
            # layer 1 (K=64 contraction over the full latent)
            hprev = []
            tsaved = []  # tsaved[l-1][m] = -h_l[m]^2
            t1 = []
            for m in range(4):
                ps = ppool.tile([128, BT], F32, tag="ps")
                nc.tensor.matmul(ps[:], w1_sb[:, m * 128:(m + 1) * 128],
                                 zr_sb[:, btsl], start=True, stop=True)
                h = hpool.tile([128, BT], F32R, tag="h")
                nc.scalar.activation(h[:], ps[:], AF.Tanh, bias=bia_sb[:, m:m + 1])
                t = tpool.tile([128, BT], F32, tag="t")
                nc.vector.scalar_tensor_tensor(t[:], h[:], -1.0, h[:], ALU.mult, ALU.mult)
                hprev.append(h)
                t1.append(t)
            tsaved.append(t1)
            # layers 2..5
            for li in range(4):
                hcur = []
                tl = []
                for m in range(4):
                    ps = ppool.tile([128, BT], F32, tag="ps")
                    for k in range(4):
                        lhsT = wf_sb[:, li * 4 * HID + k * HID + m * 128:
                                     li * 4 * HID + k * HID + (m + 1) * 128]
                        nc.tensor.matmul(ps[:], lhsT, hprev[k][:],
                                         start=(k == 0), stop=(k == 3))
                    h = hpool.tile([128, BT], F32R, tag="h")
                    nc.scalar.activation(h[:], ps[:], AF.Tanh,
                                         bias=bia_sb[:, (li + 1) * 4 + m:(li + 1) * 4 + m + 1])
                    t = tpool.tile([128, BT], F32, tag="t")
                    nc.vector.scalar_tensor_tensor(t[:], h[:], -1.0, h[:],
                                                   ALU.mult, ALU.mult)
                    hcur.append(h)
                    tl.append(t)
                tsaved.append(tl)
                hprev = hcur
            # backward seed: d5 = (1 - h5^2) * W6  (per-partition scalar W6)
            dcur = []
            for m in range(4):
                dd = dpool.tile([128, BT], F32R, tag="d")
                nc.vector.tensor_scalar(dd[:], tsaved[4][m][:], 1.0,
                                        w6_sb[:, m:m + 1], ALU.add, ALU.mult)
                dcur.append(dd)
            # backward layers 5..2: d_{l-1}[k] = (t_{l-1}[k]+1) * (W_l @ d_l)[k]
            for li in range(3, -1, -1):
                dnew = []
                for k in range(4):
                    ps = ppool.tile([128, BT], F32, tag="ps")
                    for m in range(4):
                        lhsT = wb_sb[:, li * 4 * HID + m * HID + k * 128:
                                     li * 4 * HID + m * HID + (k + 1) * 128]
                        nc.tensor.matmul(ps[:], lhsT, dcur[m][:],
                                         start=(m == 0), stop=(m == 3))
                    dd = dpool.tile([128, BT], F32R, tag="d")
                    nc.vector.scalar_tensor_tensor(dd[:], tsaved[li][k][:], 1.0,
                                                   ps[:], ALU.add, ALU.mult)
                    dnew.append(dd)
                dcur = dnew
            # swapped active gradient [4, BT]
            gps = spool.tile([4, BT], F32, tag="gps")
            for k in range(4):
                nc.tensor.matmul(gps[:], wga_sb[:, 4 * k:4 * k + 4], dcur[k][:],
                                 start=(k == 0), stop=(k == 3))
            return gps

        for bt in range(n_bt):
            btsl = slice(bt * BT, (bt + 1) * BT)
            for s in range(n_step):
                for half in range(2):
                    hs = 2 * s + half
                    gps = grad_active(btsl)
                    ga = gpool.tile([4, BT], F32, tag="ga")
                    nc.scalar.activation(ga[:], gps[:], AF.Copy)
                    # znew = cvec .* gaSwap + A_hs^T @ z_active
                    pz = spool.tile([4, BT], F32, tag="pz")
                    nc.tensor.matmul(pz[:], smp_sb[0:4, 4 * hs:4 * hs + 4],
                                     zr_sb[0:4, btsl], start=True, stop=True)
                    nc.vector.scalar_tensor_tensor(
                        zf_sb[0:4, btsl], ga[:], cvp_sb[:, hs:hs + 1], pz[:],
                        ALU.mult, ALU.add)
                    nc.vector.scalar_tensor_tensor(
                        zr_sb[0:4, btsl], ga[:], cvp_sb[:, hs:hs + 1], pz[:],
                        ALU.mult, ALU.add)
            nc.sync.dma_start(zo[:, btsl], zf_sb[:, btsl])

    nc.compile()
    return nc


def _host_prep(z, W1, b1, W2, b2, W3, b3, W4, b4, W5, b5, W6, b6, S,
               dt_q, dt_p, alpha):
    """Build the per-core input maps (weight transforms are O(HID^2) only)."""
    a1c = 1.0 / (4.0 - 4.0 ** (1.0 / 3.0))
    a3c = 1.0 - 4.0 * a1c
    dts = [a * DT for a in (a1c, a1c, a3c, a1c, a1c)]
    dtq = float(np.asarray(dt_q).reshape(-1)[0])
    dtp = float(np.asarray(dt_p).reshape(-1)[0])
    al = float(np.asarray(alpha))
    S = np.asarray(S, np.float32)

    smp = np.zeros((4, 4 * 2 * NSTEP), np.float32)
    cvp = np.zeros((4, 2 * NSTEP), np.float32)
    eye = np.eye(4, dtype=np.float32)
    for s, dt in enumerate(dts):
        cg1 = dt * dtq            # scales dH/dz2 in the z1 update
        cg2 = -(dt / 2.0) * dtp   # scales dH/dz1 in the z2 update
        A = eye.copy()
        # z1_new cols: + alpha*dt*(z@S^T)[:, :2]  -> A[i,j] += al*dt*S[j,i], j<2
        A[:, 0:2] += al * dt * S[0:2, :].T
        # z2_new cols: + alpha*(dt/2)*(z@S)[:, 2:] -> A[i,j] += al*dt/2*S[i,j], j>=2
        A[:, 2:4] += al * (dt / 2.0) * S[:, 2:4]
        Ab = eye.copy()
        Ab[:, 2:4] = A[:, 2:4]
        smp[:, 4 * (2 * s):4 * (2 * s) + 4] = A
        smp[:, 4 * (2 * s + 1):4 * (2 * s + 1) + 4] = Ab
        cvp[:, 2 * s] = [cg1, cg1, cg2, cg2]
        cvp[:, 2 * s + 1] = [0.0, 0.0, cg2, cg2]

    W1 = np.asarray(W1, np.float32)
    wga_full = W1[0:4, :].T[:, [2, 3, 0, 1]]  # [512, 4], swapped columns
    wf = np.concatenate([_pack_k(np.asarray(w, np.float32)) for w in (W2, W3, W4, W5)], axis=1)
    wb = np.concatenate([_pack_k(np.asarray(w, np.float32).T.copy()) for w in (W2, W3, W4, W5)], axis=1)
    wga = _pack_k(wga_full)
    w6p = np.asarray(W6, np.float32).reshape(4, 128).T.copy()  # [128,4], col k = W6[k*128:(k+1)*128]
    bia = np.zeros((128, 4 * NL), np.float32)
    for li, b in enumerate((b1, b2, b3, b4, b5)):
        bia[:, 4 * li:4 * li + 4] = np.asarray(b, np.float32).reshape(4, 128).T

    shared = {"w1": W1, "wf": wf, "wb": wb, "wga": wga, "w6": w6p,
              "bia": bia, "smp": smp, "cvp": cvp}
    z = np.asarray(z, np.float32)
    in_maps = []
    for c in range(N_CORES):
        zc = np.ascontiguousarray(z[c * BC:(c + 1) * BC, :].T)  # [64, 4096]
        m = dict(shared)
        m["zf"] = zc
        m["zr"] = zc
        in_maps.append(m)
    return in_maps


_cached_nc = None


def kernel(z, W1, b1, W2, b2, W3, b3, W4, b4, W5, b5, W6, b6, S,
           dt_q, dt_p, alpha, _trace=False, _trace_kwargs=None):
    global _cached_nc
    in_maps = _host_prep(z, W1, b1, W2, b2, W3, b3, W4, b4, W5, b5, W6, b6, S,
                         dt_q, dt_p, alpha)
    if _cached_nc is None:
        _cached_nc = build_program()
    nc = _cached_nc
    res = run_bass_kernel_spmd(
        nc, in_maps, core_ids=list(range(N_CORES)), trace=_trace,
        **(_trace_kwargs or {}),
    )
    kernel.last_result = res
    out = np.empty((B, LAT), np.float32)
    for c in range(N_CORES):
        out[c * BC:(c + 1) * BC, :] = res.results[c]["zo"].T
    return out


# revision 4
# speedup vs baseline: 1.1161x; 1.1161x over previous
"""Trainium2 Bass kernel for nn_ExtendedSympNet (Suzuki-4 composition of
extended symplectic verlet steps driven by a 6-layer MLP Hamiltonian).

Strategy: pure data parallel over 8 NeuronCores (4096 samples each).
Activations are kept feature-major [512 feat (partitions), 512 batch (free)];
each of the 10 gradient evaluations is a fused forward+backward pass of the
MLP done fully on-chip. Matmuls run in float32r (full PE rate at N=512),
the integrator state stays in float32.

Self-contained: hardcodes all shapes from the problem spec.
"""
import numpy as np
from contextlib import ExitStack

import concourse.bacc as bacc
import concourse.bass as bass
import concourse.mybir as mybir
import concourse.tile as tile
from concourse.bass_utils import run_bass_kernel_spmd

F32 = mybir.dt.float32
F32R = mybir.dt.float32r
AF = mybir.ActivationFunctionType
ALU = mybir.AluOpType

B, LAT, HID = 32768, 64, 512
N_CORES = 8
BC = B // N_CORES          # samples per core = 4096
BT = 512                   # batch tile (matmul moving dim / PSUM bank)
NBT = BC // BT             # 8 batch tiles per core
DT = 0.1
NSTEP = 5                  # Suzuki composition sub-steps
NL = 5                     # tanh layers in H-net


def _pack_k(w: np.ndarray) -> np.ndarray:
    """[512, C] -> [128, 4*C]: 128-row k-tile blocks side by side."""
    assert w.shape[0] == 4 * 128
    return np.concatenate([w[k * 128:(k + 1) * 128, :] for k in range(4)], axis=1)


def build_program(n_bt: int = NBT, n_step: int = NSTEP):
    nc = bacc.Bacc("TRN2", target_bir_lowering=False, debug=False)

    # ---- DRAM io ----
    d = {}
    d["zf"] = nc.dram_tensor("zf", [LAT, BC], F32, kind="ExternalInput").ap()
    d["zr"] = nc.dram_tensor("zr", [LAT, BC], F32R, kind="ExternalInput").ap()
    d["w1"] = nc.dram_tensor("w1", [LAT, HID], F32R, kind="ExternalInput").ap()
    # forward weights W2..W5 packed: [128, 4 layers * 2048]
    d["wf"] = nc.dram_tensor("wf", [128, 4 * 4 * HID], F32R, kind="ExternalInput").ap()
    # backward weights W2^T..W5^T packed the same way
    d["wb"] = nc.dram_tensor("wb", [128, 4 * 4 * HID], F32R, kind="ExternalInput").ap()
    # W1[:4,:].T with columns permuted [2,3,0,1], packed: [128, 16]
    d["wga"] = nc.dram_tensor("wga", [128, 16], F32R, kind="ExternalInput").ap()
    d["w6"] = nc.dram_tensor("w6", [128, 4], F32, kind="ExternalInput").ap()
    d["bia"] = nc.dram_tensor("bia", [128, 4 * NL], F32, kind="ExternalInput").ap()
    # per half-step active-update matrices A_hs [4,4] packed: [4, 4*2*NSTEP]
    d["smp"] = nc.dram_tensor("smp", [4, 4 * 2 * NSTEP], F32R, kind="ExternalInput").ap()
    # per half-step gradient coefficient vectors: [4, 2*NSTEP]
    d["cvp"] = nc.dram_tensor("cvp", [4, 2 * NSTEP], F32, kind="ExternalInput").ap()
    zo = nc.dram_tensor("zo", [LAT, BC], F32, kind="ExternalOutput").ap()

    with tile.TileContext(nc) as tc, ExitStack() as ctx:
        wpool = ctx.enter_context(tc.tile_pool(name="wpool", bufs=1))
        hpool = ctx.enter_context(tc.tile_pool(name="hpool", bufs=9))
        tpool = ctx.enter_context(tc.tile_pool(name="tpool", bufs=24))
        dpool = ctx.enter_context(tc.tile_pool(name="dpool", bufs=9))
        gpool = ctx.enter_context(tc.tile_pool(name="gpool", bufs=3))
        ppool = ctx.enter_context(tc.tile_pool(name="ppool", bufs=5, space="PSUM"))
        spool = ctx.enter_context(tc.tile_pool(name="spool", bufs=1, space="PSUM"))
        zpool = ctx.enter_context(tc.tile_pool(name="zpool", bufs=2, space="PSUM"))

        # ---- persistent SBUF ----
        zf_sb = wpool.tile([LAT, BC], F32)      # integrator state (output)
        zr_sb = wpool.tile([LAT, BC], F32R)     # matmul-operand mirror
        w1_sb = wpool.tile([LAT, HID], F32R)
        wf_sb = wpool.tile([128, 4 * 4 * HID], F32R)
        wb_sb = wpool.tile([128, 4 * 4 * HID], F32R)
        wga_sb = wpool.tile([128, 16], F32R)
        w6_sb = wpool.tile([128, 4], F32)
        bia_sb = wpool.tile([128, 4 * NL], F32)
        smp_sb = wpool.tile([4, 4 * 2 * NSTEP], F32R)
        cvp_sb = wpool.tile([4, 2 * NSTEP], F32)
        for name, t in (("zf", zf_sb), ("zr", zr_sb), ("w1", w1_sb), ("wf", wf_sb),
                        ("wb", wb_sb), ("wga", wga_sb), ("w6", w6_sb), ("bia", bia_sb),
                        ("smp", smp_sb), ("cvp", cvp_sb)):
            nc.sync.dma_start(t[:], d[name][:])

        def grad_active(btsl):
            """Forward+backward through the H-net for one batch tile.
            Returns the PSUM tile [4, BT] holding the SWAPPED active gradient:
            rows 0:2 = dH/dz2, rows 2:4 = dH/dz1 (at zr[:, btsl])."""
            # layer 1 (K=64 contraction over the full latent)
            hprev = []
            tsaved = []  # tsaved[l-1][m] = -h_l[m]^2
            t1 = []
            for m in range(4):
                ps = ppool.tile([128, BT], F32, tag="ps")
                nc.tensor.matmul(ps[:], w1_sb[:, m * 128:(m + 1) * 128],
                                 zr_sb[:, btsl], start=True, stop=True)
                h = hpool.tile([128, BT], F32R, tag="h")
                nc.scalar.activation(h[:], ps[:], AF.Tanh, bias=bia_sb[:, m:m + 1])
                t = tpool.tile([128, BT], F32, tag="t")
                nc.vector.scalar_tensor_tensor(t[:], h[:], -1.0, h[:], ALU.mult, ALU.mult)
                hprev.append(h)
                t1.append(t)
            tsaved.append(t1)
            # layers 2..5
            for li in range(4):
                hcur = []
                tl = []
                for m in range(4):
                    ps = ppool.tile([128, BT], F32, tag="ps")
                    for k in range(4):
                        lhsT = wf_sb[:, li * 4 * HID + k * HID + m * 128:
                                     li * 4 * HID + k * HID + (m + 1) * 128]
                        nc.tensor.matmul(ps[:], lhsT, hprev[k][:],
                                         start=(k == 0), stop=(k == 3))
                    h = hpool.tile([128, BT], F32R, tag="h")
                    nc.scalar.activation(h[:], ps[:], AF.Tanh,
                                         bias=bia_sb[:, (li + 1) * 4 + m:(li + 1) * 4 + m + 1])
                    t = tpool.tile([128, BT], F32, tag="t")
                    nc.vector.scalar_tensor_tensor(t[:], h[:], -1.0, h[:],
                                                   ALU.mult, ALU.mult)
                    hcur.append(h)
                    tl.append(t)
                tsaved.append(tl)
                hprev = hcur
            # backward seed: d5 = (1 - h5^2) * W6  (per-partition scalar W6)
            dcur = []
            for m in range(4):
                dd = dpool.tile([128, BT], F32R, tag="d")
                nc.vector.tensor_scalar(dd[:], tsaved[4][m][:], 1.0,
                                        w6_sb[:, m:m + 1], ALU.add, ALU.mult)
                dcur.append(dd)
            # backward layers 5..2: d_{l-1}[k] = (t_{l-1}[k]+1) * (W_l @ d_l)[k]
            for li in range(3, -1, -1):
                dnew = []
                for k in range(4):
                    ps = ppool.tile([128, BT], F32, tag="ps")
                    for m in range(4):
                        lhsT = wb_sb[:, li * 4 * HID + m * HID + k * 128:
                                     li * 4 * HID + m * HID + (k + 1) * 128]
                        nc.tensor.matmul(ps[:], lhsT, dcur[m][:],
                                         start=(m == 0), stop=(m == 3))
                    dd = dpool.tile([128, BT], F32R, tag="d")
                    nc.vector.scalar_tensor_tensor(dd[:], tsaved[li][k][:], 1.0,
                                                   ps[:], ALU.add, ALU.mult)
                    dnew.append(dd)
                dcur = dnew
            # swapped active gradient [4, BT]
            gps = spool.tile([4, BT], F32, tag="gps")
            for k in range(4):
                nc.tensor.matmul(gps[:], wga_sb[:, 4 * k:4 * k + 4], dcur[k][:],
                                 start=(k == 0), stop=(k == 3))
            return gps

        def half_step(btsl, hs):
            """One half-step: gradient, then active-dim update (state + mirror)."""
            gps = grad_active(btsl)
            ga = gpool.tile([4, BT], F32, tag="ga")
            nc.scalar.activation(ga[:], gps[:], AF.Copy)
            # znew = cvec .* gaSwap + A_hs^T @ z_active
            pz = zpool.tile([4, BT], F32, tag="pz")
            nc.tensor.matmul(pz[:], smp_sb[0:4, 4 * hs:4 * hs + 4],
                             zr_sb[0:4, btsl], start=True, stop=True)
            nc.vector.scalar_tensor_tensor(
                zf_sb[0:4, btsl], ga[:], cvp_sb[:, hs:hs + 1], pz[:],
                ALU.mult, ALU.add)
            nc.vector.scalar_tensor_tensor(
                zr_sb[0:4, btsl], ga[:], cvp_sb[:, hs:hs + 1], pz[:],
                ALU.mult, ALU.add)

        # interleave pairs of independent batch-tile chains so the PE can fill
        # each chain's update-dependency bubble with the other chain's matmuls
        for p in range(0, n_bt, 2):
            slA = slice(p * BT, (p + 1) * BT)
            pair = p + 1 < n_bt
            slB = slice((p + 1) * BT, (p + 2) * BT) if pair else None
            for s in range(n_step):
                for half in range(2):
                    hs = 2 * s + half
                    half_step(slA, hs)
                    if pair:
                        half_step(slB, hs)
            nc.sync.dma_start(zo[:, slA], zf_sb[:, slA])
            if pair:
                nc.sync.dma_start(zo[:, slB], zf_sb[:, slB])

    nc.compile()
    return nc


def _host_prep(z, W1, b1, W2, b2, W3, b3, W4, b4, W5, b5, W6, b6, S,
               dt_q, dt_p, alpha):
    """Build the per-core input maps (weight transforms are O(HID^2) only)."""
    a1c = 1.0 / (4.0 - 4.0 ** (1.0 / 3.0))
    a3c = 1.0 - 4.0 * a1c
    dts = [a * DT for a in (a1c, a1c, a3c, a1c, a1c)]
    dtq = float(np.asarray(dt_q).reshape(-1)[0])
    dtp = float(np.asarray(dt_p).reshape(-1)[0])
    al = float(np.asarray(alpha))
    S = np.asarray(S, np.float32)

    smp = np.zeros((4, 4 * 2 * NSTEP), np.float32)
    cvp = np.zeros((4, 2 * NSTEP), np.float32)
    eye = np.eye(4, dtype=np.float32)
    for s, dt in enumerate(dts):
        cg1 = dt * dtq            # scales dH/dz2 in the z1 update
        cg2 = -(dt / 2.0) * dtp   # scales dH/dz1 in the z2 update
        A = eye.copy()
        # z1_new cols: + alpha*dt*(z@S^T)[:, :2]  -> A[i,j] += al*dt*S[j,i], j<2
        A[:, 0:2] += al * dt * S[0:2, :].T
        # z2_new cols: + alpha*(dt/2)*(z@S)[:, 2:] -> A[i,j] += al*dt/2*S[i,j], j>=2
        A[:, 2:4] += al * (dt / 2.0) * S[:, 2:4]
        Ab = eye.copy()
        Ab[:, 2:4] = A[:, 2:4]
        smp[:, 4 * (2 * s):4 * (2 * s) + 4] = A
        smp[:, 4 * (2 * s + 1):4 * (2 * s + 1) + 4] = Ab
        cvp[:, 2 * s] = [cg1, cg1, cg2, cg2]
        cvp[:, 2 * s + 1] = [0.0, 0.0, cg2, cg2]

    W1 = np.asarray(W1, np.float32)
    wga_full = W1[0:4, :].T[:, [2, 3, 0, 1]]  # [512, 4], swapped columns
    wf = np.concatenate([_pack_k(np.asarray(w, np.float32)) for w in (W2, W3, W4, W5)], axis=1)
    wb = np.concatenate([_pack_k(np.asarray(w, np.float32).T.copy()) for w in (W2, W3, W4, W5)], axis=1)
    wga = _pack_k(wga_full)
    w6p = np.asarray(W6, np.float32).reshape(4, 128).T.copy()  # [128,4], col k = W6[k*128:(k+1)*128]
    bia = np.zeros((128, 4 * NL), np.float32)
    for li, b in enumerate((b1, b2, b3, b4, b5)):
        bia[:, 4 * li:4 * li + 4] = np.asarray(b, np.float32).reshape(4, 128).T

    shared = {"w1": W1, "wf": wf, "wb": wb, "wga": wga, "w6": w6p,
              "bia": bia, "smp": smp, "cvp": cvp}
    z = np.asarray(z, np.float32)
    in_maps = []
    for c in range(N_CORES):
        zc = np.ascontiguousarray(z[c * BC:(c + 1) * BC, :].T)  # [64, 4096]
        m = dict(shared)
        m["zf"] = zc
        m["zr"] = zc
        in_maps.append(m)
    return in_maps


_cached_nc = None


def kernel(z, W1, b1, W2, b2, W3, b3, W4, b4, W5, b5, W6, b6, S,
           dt_q, dt_p, alpha, _trace=False, _trace_kwargs=None):
    global _cached_nc
    in_maps = _host_prep(z, W1, b1, W2, b2, W3, b3, W4, b4, W5, b5, W6, b6, S,
                         dt_q, dt_p, alpha)
    if _cached_nc is None:
        _cached_nc = build_program()
    nc = _cached_nc
    res = run_bass_kernel_spmd(
        nc, in_maps, core_ids=list(range(N_CORES)), trace=_trace,
        **(_trace_kwargs or {}),
    )
    kernel.last_result = res
    out = np.empty((B, LAT), np.float32)
    for c in range(N_CORES):
        out[c * BC:(c + 1) * BC, :] = res.results[c]["zo"].T
    return out
